# revision 15
# baseline (speedup 1.0000x reference)
"""Trainium2 Bass kernel for nn_AttentionLayer (single-query attention).

Reference computation (per batch b):
    sub  = dec[b] @ W.T                  # [512]
    s    = enc[b] @ sub                  # [4096] scores
    attn = softmax(s)                    # [4096]
    out  = attn @ enc[b]                 # [512]
Returns (attn [B,1,S], sumResult [B,E]).

Sharding: data-parallel over batch. 8 cores x 8 batches each, W replicated.
Each core streams its 64 MB of encoderFeature through SBUF exactly once
(memory-bound); scores are computed on VectorE with the fused
scalar_tensor_tensor (multiply + free-dim accumulate) op, the weighted sum
on TensorE in bf16 from a ScalarE-cast copy, softmax plumbing on
ScalarE/TensorE/VectorE.
"""

import numpy as np
from contextlib import ExitStack

import concourse.bass as bass
import concourse.bacc as bacc
import concourse.tile as tile
from concourse import mybir
from concourse.bass_utils import run_bass_kernel_spmd

F32 = mybir.dt.float32
BF16 = mybir.dt.bfloat16

# Problem shape (hardcoded; kernel.py must be self-contained).
B, S, E = 64, 4096, 512
NCORES = 8
NB = B // NCORES          # local batches per core
P = 128                   # partitions
NJ = S // P               # 32 score columns per batch
NCHUNK = 4                # DMA chunks per batch
JPC = NJ // NCHUNK        # 8 columns per chunk
OFF = 3                   # score columns per batch computed on TensorE


def build_nc(level=3):
    nc = bacc.Bacc("TRN2", target_bir_lowering=False, debug=False)

    dect_d = nc.dram_tensor("dect_in", [E, NB], F32, kind="ExternalInput")
    enc_d = nc.dram_tensor("enc_in", [NB, S, E], F32, kind="ExternalInput")
    wt_d = nc.dram_tensor("wt_in", [E, E], F32, kind="ExternalInput")
    id_d = nc.dram_tensor("ident_in", [P, P], F32, kind="ExternalInput")
    attn_d = nc.dram_tensor("attn_out", [NB, S], F32, kind="ExternalOutput")
    sum_d = nc.dram_tensor("sum_out", [NB, E], F32, kind="ExternalOutput")

    with tile.TileContext(nc) as tc, ExitStack() as ctx:
        const_p = ctx.enter_context(tc.tile_pool(name="const", bufs=1))
        set_ps = ctx.enter_context(tc.tile_pool(name="set_ps", bufs=2, space="PSUM"))
        set_sb = ctx.enter_context(tc.tile_pool(name="set_sb", bufs=2))
        enc_p = ctx.enter_context(tc.tile_pool(name="enc_p", bufs=5))
        encb_p = ctx.enter_context(tc.tile_pool(name="encb_p", bufs=2 * NCHUNK))
        sm_p = ctx.enter_context(tc.tile_pool(name="sm_p", bufs=2))
        small_p = ctx.enter_context(tc.tile_pool(name="small_p", bufs=2))
        ps_p = ctx.enter_context(tc.tile_pool(name="ps_p", bufs=2, space="PSUM"))
        sum_ps = ctx.enter_context(tc.tile_pool(name="sum_ps", bufs=2, space="PSUM"))
        et_ps = ctx.enter_context(tc.tile_pool(name="et_ps", bufs=2, space="PSUM"))
        et_sb = ctx.enter_context(tc.tile_pool(name="et_sb", bufs=3))

        # ---- constants ----
        ident = const_p.tile([P, P], F32)
        nc.sync.dma_start(out=ident, in_=id_d.ap())
        ones_row = const_p.tile([1, P], F32)
        nc.vector.memset(ones_row, 1.0)
        ones_col = const_p.tile([P, 1], F32)
        nc.vector.memset(ones_col, 1.0)
        dummy = const_p.tile([P, 1], F32)

        # ---- input DMAs for W^T / dec^T, then batch-0 enc prefetch ----
        wt_sb = const_p.tile([P, 4, E], F32)  # [d within chunk j, j, e']
        nc.sync.dma_start(out=wt_sb,
                          in_=wt_d.ap().rearrange("(j p) e -> p j e", p=P))
        dect_sb = const_p.tile([P, 4, NB], F32)  # [d within chunk j, j, b]
        nc.sync.dma_start(out=dect_sb,
                          in_=dect_d.ap().rearrange("(j p) b -> p j b", p=P))

        batch_tiles = [None] * NB

        def issue_load(b):
            enc_b = enc_d.ap()[b].rearrange("(p j) e -> p j e", p=P)  # s = 32p + j
            chunks, chunks16 = [], []
            for c in range(NCHUNK):
                et = enc_p.tile([P, JPC, E], F32, tag="enc")
                nc.sync.dma_start(out=et, in_=enc_b[:, c * JPC:(c + 1) * JPC, :])
                chunks.append(et)
                if level >= 3:
                    ebt = encb_p.tile([P, JPC, E], BF16, tag="encb")
                    nc.scalar.copy(out=ebt, in_=et)
                    chunks16.append(ebt)
            batch_tiles[b] = (chunks, chunks16)

        issue_load(0)

        # ---- setup: sub = dec @ W^T rows, broadcast + transposed copies ----
        subb = []
        subT = []
        if level == 0:
            for b in range(NB):
                sb = const_p.tile([P, E], F32, tag=f"subb{b}")
                nc.vector.memset(sb, 0.01)
                subb.append(sb)
                stp = const_p.tile([P, 4], F32, tag=f"subT{b}")
                nc.vector.memset(stp, 0.01)
                subT.append(stp)
        else:
            # sub_b = dec[b] @ W^T as a [1, E] row, broadcast to all partitions
            for b in range(NB):
                sub_b_ps = set_ps.tile([1, E], F32, tag="s")
                for j in range(4):
                    nc.tensor.matmul(sub_b_ps, dect_sb[:, j, b:b + 1],
                                     wt_sb[:, j, :], start=(j == 0), stop=(j == 3))
                sub_b = set_sb.tile([1, E], F32, tag="sub_b")
                nc.vector.tensor_copy(out=sub_b, in_=sub_b_ps)
                psb = set_ps.tile([P, E], F32, tag="s")
                nc.tensor.matmul(psb, ones_row, sub_b, start=True, stop=True)
                sb = const_p.tile([P, E], F32, tag=f"subb{b}")
                nc.vector.tensor_copy(out=sb, in_=psb)
                subb.append(sb)

            # subT[b] [128, 4]: sub with e' on partitions, for the PE-path
            for b in range(NB):
                stp = const_p.tile([P, 4], F32, tag=f"subT{b}")
                for ec in range(4):
                    tps = ps_p.tile([P, 1], F32, tag="sp")
                    nc.tensor.transpose(
                        tps, subb[b][0:1, ec * P:(ec + 1) * P], ident[0:1, 0:1])
                    nc.scalar.copy(out=stp[:, ec:ec + 1], in_=tps)
                subT.append(stp)

        # ---- main loop over local batches ----
        for b in range(NB):
            if b + 1 < NB:
                issue_load(b + 1)
            chunks, chunks16 = batch_tiles[b]

            # pass 1: scores[p, j] = enc_row . sub
            # DVE (fused mult+accum) for most columns, PE path for the last OFF
            scores = sm_p.tile([P, NJ], F32, tag="scores")
            for c in range(NCHUNK):
                for jj in range(JPC):
                    j = c * JPC + jj
                    if j < OFF and level >= 2:
                        continue
                    nc.vector.scalar_tensor_tensor(
                        out=dummy.broadcast_to((P, E)),
                        in0=chunks[c][:, jj, :],
                        scalar=0.0,
                        in1=subb[b],
                        op0=mybir.AluOpType.bypass,
                        op1=mybir.AluOpType.mult,
                        accum_out=scores[:, j:j + 1],
                    )
            if level >= 2:
                for j in range(OFF):
                    c, jj = j // JPC, j % JPC
                    etp = et_ps.tile([P, 4, P], F32, tag="etp")
                    for ec in range(4):
                        nc.tensor.transpose(
                            etp[:, ec, :],
                            chunks[c][:, jj, ec * P:(ec + 1) * P], ident)
                    ets = et_sb.tile([P, 4, P], F32, tag="ets")
                    nc.scalar.copy(out=ets, in_=etp)
                    scol = ps_p.tile([P, 1], F32, tag="sp")
                    for ec in range(4):
                        nc.tensor.matmul(scol, ets[:, ec, :],
                                         subT[b][:, ec:ec + 1],
                                         start=(ec == 0), stop=(ec == 3))
                    nc.scalar.copy(out=scores[:, j:j + 1], in_=scol)

            if level <= 1:
                nc.scalar.dma_start(
                    out=attn_d.ap()[b].rearrange("(p j) -> p j", p=P), in_=scores)
                continue

            # softmax: global max (free reduce, PE transpose, free reduce)
            mpart = small_p.tile([P, 1], F32, tag="mpart")
            nc.vector.tensor_reduce(mpart, scores, axis=mybir.AxisListType.X,
                                    op=mybir.AluOpType.max)
            mt_ps = ps_p.tile([1, P], F32, tag="sp")
            nc.tensor.transpose(mt_ps, mpart, ident)
            negm11 = small_p.tile([1, 1], F32, tag="negm11")
            nc.vector.tensor_reduce(negm11, mt_ps, axis=mybir.AxisListType.X,
                                    op=mybir.AluOpType.max, negate=True)
            negm_ps = ps_p.tile([P, 1], F32, tag="sp")
            nc.tensor.matmul(negm_ps, ones_row, negm11, start=True, stop=True)
            negm = small_p.tile([P, 1], F32, tag="negm")
            nc.scalar.copy(out=negm, in_=negm_ps)

            # exp(scores - max) with fused per-partition denominator
            p_sb = sm_p.tile([P, NJ], F32, tag="p")
            sumexp = small_p.tile([P, 1], F32, tag="sumexp")
            nc.scalar.activation(out=p_sb, in_=scores,
                                 func=mybir.ActivationFunctionType.Exp,
                                 bias=negm, scale=1.0, accum_out=sumexp)

            # Z = sum over partitions; rZ = 1/Z broadcast back
            z_ps = ps_p.tile([1, 1], F32, tag="sp")
            nc.tensor.matmul(z_ps, sumexp, ones_col, start=True, stop=True)
            rz11 = small_p.tile([1, 1], F32, tag="rz11")
            nc.vector.reciprocal(rz11, z_ps)
            rz_ps = ps_p.tile([P, 1], F32, tag="sp")
            nc.tensor.matmul(rz_ps, ones_row, rz11, start=True, stop=True)
            rz = small_p.tile([P, 1], F32, tag="rz")
            nc.scalar.copy(out=rz, in_=rz_ps)

            # normalized attention out
            attn_n = sm_p.tile([P, NJ], F32, tag="attn_n")
            nc.vector.tensor_scalar_mul(attn_n, p_sb, rz)
            nc.scalar.dma_start(out=attn_d.ap()[b].rearrange("(p j) -> p j", p=P),
                                in_=attn_n)

            if level <= 2:
                continue

            # pass 2: sumResult = (sum_j exp_col_j^T @ enc_col_j) / Z on PE (bf16)
            p16 = sm_p.tile([P, NJ], BF16, tag="p16")
            nc.scalar.copy(out=p16, in_=p_sb)
            sres_ps = sum_ps.tile([1, E], F32, tag="sres")
            for c in range(NCHUNK):
                for jj in range(JPC):
                    j = c * JPC + jj
                    nc.tensor.matmul(sres_ps, p16[:, j:j + 1], chunks16[c][:, jj, :],
                                     start=(j == 0), stop=(j == NJ - 1))
            sres = small_p.tile([1, E], F32, tag="sres_sb")
            nc.scalar.activation(out=sres, in_=sres_ps,
                                 func=mybir.ActivationFunctionType.Copy,
                                 bias=0.0, scale=rz11)
            nc.scalar.dma_start(out=sum_d.ap()[b], in_=sres)

    nc.compile()
    return nc


_NC_CACHE = None


def _get_nc():
    global _NC_CACHE
    if _NC_CACHE is None:
        _NC_CACHE = build_nc()
    return _NC_CACHE


def kernel(decoderFeature, encoderFeature, W, **_ignored):
    """Full inputs in, full outputs out. Shards batch across 8 NeuronCores."""
    dec = np.ascontiguousarray(np.asarray(decoderFeature, dtype=np.float32)
                               .reshape(B, E))
    enc = np.ascontiguousarray(np.asarray(encoderFeature, dtype=np.float32))
    w = np.ascontiguousarray(np.asarray(W, dtype=np.float32))

    wt = np.ascontiguousarray(w.T)
    ident = np.eye(P, dtype=np.float32)
    nc = _get_nc()
    in_maps = []
    for c in range(NCORES):
        lo, hi = c * NB, (c + 1) * NB
        in_maps.append({
            "dect_in": np.ascontiguousarray(dec[lo:hi].T),
            "enc_in": np.ascontiguousarray(enc[lo:hi]),
            "wt_in": wt,
            "ident_in": ident,
        })
    res = run_bass_kernel_spmd(nc, in_maps, core_ids=list(range(NCORES)))
    outs = res.results

    attn = np.concatenate([outs[c]["attn_out"] for c in range(NCORES)], axis=0)
    sumr = np.concatenate([outs[c]["sum_out"] for c in range(NCORES)], axis=0)
    return attn.reshape(B, 1, S).astype(np.float32), sumr.astype(np.float32)


# revision 16
# speedup vs baseline: 1.1031x; 1.1031x over previous
"""Trainium2 Bass kernel for nn_AttentionLayer (single-query attention).

Reference computation (per batch b):
    sub  = dec[b] @ W.T                  # [512]
    s    = enc[b] @ sub                  # [4096] scores
    attn = softmax(s)                    # [4096]
    out  = attn @ enc[b]                 # [512]
Returns (attn [B,1,S], sumResult [B,E]).

Sharding: data-parallel over batch. 8 cores x 8 batches each, W replicated.
Each core streams its 64 MB of encoderFeature through SBUF exactly once
(memory-bound); scores are computed on VectorE with the fused
scalar_tensor_tensor (multiply + free-dim accumulate) op, the weighted sum
on TensorE in bf16 from a ScalarE-cast copy, softmax plumbing on
ScalarE/TensorE/VectorE.
"""

import numpy as np
from contextlib import ExitStack

import concourse.bass as bass
import concourse.bacc as bacc
import concourse.tile as tile
from concourse import mybir
from concourse.bass_utils import run_bass_kernel_spmd

F32 = mybir.dt.float32
BF16 = mybir.dt.bfloat16

# Problem shape (hardcoded; kernel.py must be self-contained).
B, S, E = 64, 4096, 512
NCORES = 8
NB = B // NCORES          # local batches per core
P = 128                   # partitions
NJ = S // P               # 32 score columns per batch
NCHUNK = 4                # DMA chunks per batch
JPC = NJ // NCHUNK        # 8 columns per chunk
OFF = 0                   # score columns per batch computed on TensorE


def build_nc(level=3):
    nc = bacc.Bacc("TRN2", target_bir_lowering=False, debug=False)

    dect_d = nc.dram_tensor("dect_in", [E, NB], F32, kind="ExternalInput")
    enc_d = nc.dram_tensor("enc_in", [NB, S, E], F32, kind="ExternalInput")
    wt_d = nc.dram_tensor("wt_in", [E, E], F32, kind="ExternalInput")
    id_d = nc.dram_tensor("ident_in", [P, P], F32, kind="ExternalInput")
    attn_d = nc.dram_tensor("attn_out", [NB, S], F32, kind="ExternalOutput")
    sum_d = nc.dram_tensor("sum_out", [NB, E], F32, kind="ExternalOutput")

    with tile.TileContext(nc) as tc, ExitStack() as ctx:
        const_p = ctx.enter_context(tc.tile_pool(name="const", bufs=1))
        set_ps = ctx.enter_context(tc.tile_pool(name="set_ps", bufs=2, space="PSUM"))
        set_sb = ctx.enter_context(tc.tile_pool(name="set_sb", bufs=2))
        enc_p = ctx.enter_context(tc.tile_pool(name="enc_p", bufs=5))
        encb_p = ctx.enter_context(tc.tile_pool(name="encb_p", bufs=2 * NCHUNK))
        sm_p = ctx.enter_context(tc.tile_pool(name="sm_p", bufs=2))
        small_p = ctx.enter_context(tc.tile_pool(name="small_p", bufs=2))
        ps_p = ctx.enter_context(tc.tile_pool(name="ps_p", bufs=2, space="PSUM"))
        sum_ps = ctx.enter_context(tc.tile_pool(name="sum_ps", bufs=2, space="PSUM"))
        et_ps = ctx.enter_context(tc.tile_pool(name="et_ps", bufs=2, space="PSUM"))
        et_sb = ctx.enter_context(tc.tile_pool(name="et_sb", bufs=3))

        # ---- constants ----
        ident = const_p.tile([P, P], F32)
        ones_row = const_p.tile([1, P], F32)
        nc.vector.memset(ones_row, 1.0)
        ones_col = const_p.tile([P, 1], F32)
        nc.vector.memset(ones_col, 1.0)
        dummy = const_p.tile([P, 1], F32)

        # ---- input DMAs for W^T / dec^T, then batch-0 enc prefetch ----
        wt_sb = const_p.tile([P, 4, E], F32)  # [d within chunk j, j, e']
        nc.sync.dma_start(out=wt_sb,
                          in_=wt_d.ap().rearrange("(j p) e -> p j e", p=P))
        dect_sb = const_p.tile([P, 4, NB], F32)  # [d within chunk j, j, b]
        nc.sync.dma_start(out=dect_sb,
                          in_=dect_d.ap().rearrange("(j p) b -> p j b", p=P))
        nc.sync.dma_start(out=ident, in_=id_d.ap())

        batch_tiles = [None] * NB

        def issue_load(b):
            enc_b = enc_d.ap()[b].rearrange("(p j) e -> p j e", p=P)  # s = 32p + j
            chunks, chunks16 = [], []
            for c in range(NCHUNK):
                et = enc_p.tile([P, JPC, E], F32, tag="enc")
                if b == 0:
                    h = JPC // 2
                    nc.sync.dma_start(out=et[:, 0:h, :],
                                      in_=enc_b[:, c * JPC:c * JPC + h, :])
                    nc.sync.dma_start(out=et[:, h:JPC, :],
                                      in_=enc_b[:, c * JPC + h:(c + 1) * JPC, :])
                else:
                    nc.sync.dma_start(out=et,
                                      in_=enc_b[:, c * JPC:(c + 1) * JPC, :])
                chunks.append(et)
                if level >= 3:
                    ebt = encb_p.tile([P, JPC, E], BF16, tag="encb")
                    nc.scalar.copy(out=ebt, in_=et)
                    chunks16.append(ebt)
            batch_tiles[b] = (chunks, chunks16)

        issue_load(0)

        # ---- setup: sub = dec @ W^T rows, broadcast + transposed copies ----
        subb = []
        subT = []
        if level == 0:
            for b in range(NB):
                sb = const_p.tile([P, E], F32, tag=f"subb{b}")
                nc.vector.memset(sb, 0.01)
                subb.append(sb)
                stp = const_p.tile([P, 4], F32, tag=f"subT{b}")
                nc.vector.memset(stp, 0.01)
                subT.append(stp)
        else:
            # sub_b = dec[b] @ W^T as a [1, E] row, broadcast to all partitions
            for b in range(NB):
                sub_b_ps = set_ps.tile([1, E], F32, tag="s")
                for j in range(4):
                    nc.tensor.matmul(sub_b_ps, dect_sb[:, j, b:b + 1],
                                     wt_sb[:, j, :], start=(j == 0), stop=(j == 3))
                sub_b = set_sb.tile([1, E], F32, tag="sub_b")
                nc.vector.tensor_copy(out=sub_b, in_=sub_b_ps)
                psb = set_ps.tile([P, E], F32, tag="s")
                nc.tensor.matmul(psb, ones_row, sub_b, start=True, stop=True)
                sb = const_p.tile([P, E], F32, tag=f"subb{b}")
                nc.vector.tensor_copy(out=sb, in_=psb)
                subb.append(sb)

            # subT[b] [128, 4]: sub with e' on partitions, for the PE-path
            for b in range(NB) if OFF else []:
                stp = const_p.tile([P, 4], F32, tag=f"subT{b}")
                for ec in range(4):
                    tps = ps_p.tile([P, 1], F32, tag="sp")
                    nc.tensor.transpose(
                        tps, subb[b][0:1, ec * P:(ec + 1) * P], ident[0:1, 0:1])
                    nc.scalar.copy(out=stp[:, ec:ec + 1], in_=tps)
                subT.append(stp)

        # ---- main loop over local batches ----
        for b in range(NB):
            if b + 1 < NB:
                issue_load(b + 1)
            chunks, chunks16 = batch_tiles[b]

            # pass 1: scores[p, j] = enc_row . sub
            # DVE (fused mult+accum) for most columns, PE path for the last OFF
            scores = sm_p.tile([P, NJ], F32, tag="scores")
            for c in range(NCHUNK):
                for jj in range(JPC):
                    j = c * JPC + jj
                    if j < OFF and level >= 2:
                        continue
                    nc.vector.scalar_tensor_tensor(
                        out=dummy.broadcast_to((P, E)),
                        in0=chunks[c][:, jj, :],
                        scalar=0.0,
                        in1=subb[b],
                        op0=mybir.AluOpType.bypass,
                        op1=mybir.AluOpType.mult,
                        accum_out=scores[:, j:j + 1],
                    )
            if level >= 2:
                for j in range(OFF):
                    c, jj = j // JPC, j % JPC
                    etp = et_ps.tile([P, 4, P], F32, tag="etp")
                    for ec in range(4):
                        nc.tensor.transpose(
                            etp[:, ec, :],
                            chunks[c][:, jj, ec * P:(ec + 1) * P], ident)
                    ets = et_sb.tile([P, 4, P], F32, tag="ets")
                    nc.scalar.copy(out=ets, in_=etp)
                    scol = ps_p.tile([P, 1], F32, tag="sp")
                    for ec in range(4):
                        nc.tensor.matmul(scol, ets[:, ec, :],
                                         subT[b][:, ec:ec + 1],
                                         start=(ec == 0), stop=(ec == 3))
                    nc.scalar.copy(out=scores[:, j:j + 1], in_=scol)

            if level <= 1:
                nc.scalar.dma_start(
                    out=attn_d.ap()[b].rearrange("(p j) -> p j", p=P), in_=scores)
                continue

            # softmax: global max (free reduce, PE transpose, free reduce)
            mpart = small_p.tile([P, 1], F32, tag="mpart")
            nc.vector.tensor_reduce(mpart, scores, axis=mybir.AxisListType.X,
                                    op=mybir.AluOpType.max)
            mt_ps = ps_p.tile([1, P], F32, tag="sp")
            nc.tensor.transpose(mt_ps, mpart, ident)
            negm11 = small_p.tile([1, 1], F32, tag="negm11")
            nc.vector.tensor_reduce(negm11, mt_ps, axis=mybir.AxisListType.X,
                                    op=mybir.AluOpType.max, negate=True)
            negm_ps = ps_p.tile([P, 1], F32, tag="sp")
            nc.tensor.matmul(negm_ps, ones_row, negm11, start=True, stop=True)
            negm = small_p.tile([P, 1], F32, tag="negm")
            nc.scalar.copy(out=negm, in_=negm_ps)

            # exp(scores - max) with fused per-partition denominator
            p_sb = sm_p.tile([P, NJ], F32, tag="p")
            sumexp = small_p.tile([P, 1], F32, tag="sumexp")
            nc.scalar.activation(out=p_sb, in_=scores,
                                 func=mybir.ActivationFunctionType.Exp,
                                 bias=negm, scale=1.0, accum_out=sumexp)

            # Z = sum over partitions; rZ = 1/Z broadcast back
            z_ps = ps_p.tile([1, 1], F32, tag="sp")
            nc.tensor.matmul(z_ps, sumexp, ones_col, start=True, stop=True)
            rz11 = small_p.tile([1, 1], F32, tag="rz11")
            nc.vector.reciprocal(rz11, z_ps)
            rz_ps = ps_p.tile([P, 1], F32, tag="sp")
            nc.tensor.matmul(rz_ps, ones_row, rz11, start=True, stop=True)
            rz = small_p.tile([P, 1], F32, tag="rz")
            nc.scalar.copy(out=rz, in_=rz_ps)

            # normalized attention out
            attn_n = sm_p.tile([P, NJ], F32, tag="attn_n")
            nc.vector.tensor_scalar_mul(attn_n, p_sb, rz)
            nc.scalar.dma_start(out=attn_d.ap()[b].rearrange("(p j) -> p j", p=P),
                                in_=attn_n)

            if level <= 2:
                continue

            # pass 2: sumResult = (sum_j exp_col_j^T @ enc_col_j) / Z on PE (bf16)
            p16 = sm_p.tile([P, NJ], BF16, tag="p16")
            nc.scalar.copy(out=p16, in_=p_sb)
            sres_ps = sum_ps.tile([1, E], F32, tag="sres")
            for c in range(NCHUNK):
                for jj in range(JPC):
                    j = c * JPC + jj
                    nc.tensor.matmul(sres_ps, p16[:, j:j + 1], chunks16[c][:, jj, :],
                                     start=(j == 0), stop=(j == NJ - 1))
            sres = small_p.tile([1, E], F32, tag="sres_sb")
            nc.scalar.activation(out=sres, in_=sres_ps,
                                 func=mybir.ActivationFunctionType.Copy,
                                 bias=0.0, scale=rz11)
            nc.scalar.dma_start(out=sum_d.ap()[b], in_=sres)

    nc.compile()
    return nc


_NC_CACHE = None


def _get_nc():
    global _NC_CACHE
    if _NC_CACHE is None:
        _NC_CACHE = build_nc()
    return _NC_CACHE


def kernel(decoderFeature, encoderFeature, W, **_ignored):
    """Full inputs in, full outputs out. Shards batch across 8 NeuronCores."""
    dec = np.ascontiguousarray(np.asarray(decoderFeature, dtype=np.float32)
                               .reshape(B, E))
    enc = np.ascontiguousarray(np.asarray(encoderFeature, dtype=np.float32))
    w = np.ascontiguousarray(np.asarray(W, dtype=np.float32))

    wt = np.ascontiguousarray(w.T)
    ident = np.eye(P, dtype=np.float32)
    nc = _get_nc()
    in_maps = []
    for c in range(NCORES):
        lo, hi = c * NB, (c + 1) * NB
        in_maps.append({
            "dect_in": np.ascontiguousarray(dec[lo:hi].T),
            "enc_in": np.ascontiguousarray(enc[lo:hi]),
            "wt_in": wt,
            "ident_in": ident,
        })
    res = run_bass_kernel_spmd(nc, in_maps, core_ids=list(range(NCORES)))
    outs = res.results

    attn = np.concatenate([outs[c]["attn_out"] for c in range(NCORES)], axis=0)
    sumr = np.concatenate([outs[c]["sum_out"] for c in range(NCORES)], axis=0)
    return attn.reshape(B, 1, S).astype(np.float32), sumr.astype(np.float32)


# revision 17
# speedup vs baseline: 1.1162x; 1.0118x over previous
"""Trainium2 Bass kernel for nn_AttentionLayer (single-query attention).

Reference computation (per batch b):
    sub  = dec[b] @ W.T                  # [512]
    s    = enc[b] @ sub                  # [4096] scores
    attn = softmax(s)                    # [4096]
    out  = attn @ enc[b]                 # [512]
Returns (attn [B,1,S], sumResult [B,E]).

Sharding: data-parallel over batch. 8 cores x 8 batches each, W replicated.
Each core streams its 64 MB of encoderFeature through SBUF exactly once
(memory-bound); scores are computed on VectorE with the fused
scalar_tensor_tensor (multiply + free-dim accumulate) op, the weighted sum
on TensorE in bf16 from a ScalarE-cast copy, softmax plumbing on
ScalarE/TensorE/VectorE.
"""

import numpy as np
from contextlib import ExitStack

import concourse.bass as bass
import concourse.bacc as bacc
import concourse.tile as tile
from concourse import mybir
from concourse.bass_utils import run_bass_kernel_spmd

F32 = mybir.dt.float32
BF16 = mybir.dt.bfloat16

# Problem shape (hardcoded; kernel.py must be self-contained).
B, S, E = 64, 4096, 512
NCORES = 8
NB = B // NCORES          # local batches per core
P = 128                   # partitions
NJ = S // P               # 32 score columns per batch
NCHUNK = 4                # DMA chunks per batch
JPC = NJ // NCHUNK        # 8 columns per chunk
OFF = 0                   # score columns per batch computed on TensorE


def build_nc(level=3):
    nc = bacc.Bacc("TRN2", target_bir_lowering=False, debug=False)

    dect_d = nc.dram_tensor("dect_in", [E, NB], F32, kind="ExternalInput")
    enc_d = nc.dram_tensor("enc_in", [NB, S, E], F32, kind="ExternalInput")
    wt_d = nc.dram_tensor("wt_in", [E, E], F32, kind="ExternalInput")
    id_d = nc.dram_tensor("ident_in", [P, P], F32, kind="ExternalInput")
    attn_d = nc.dram_tensor("attn_out", [NB, S], F32, kind="ExternalOutput")
    sum_d = nc.dram_tensor("sum_out", [NB, E], F32, kind="ExternalOutput")

    with tile.TileContext(nc) as tc, ExitStack() as ctx:
        const_p = ctx.enter_context(tc.tile_pool(name="const", bufs=1))
        set_ps = ctx.enter_context(tc.tile_pool(name="set_ps", bufs=2, space="PSUM"))
        set_sb = ctx.enter_context(tc.tile_pool(name="set_sb", bufs=2))
        enc_p = ctx.enter_context(tc.tile_pool(name="enc_p", bufs=5))
        encb_p = ctx.enter_context(tc.tile_pool(name="encb_p", bufs=2 * NCHUNK))
        sm_p = ctx.enter_context(tc.tile_pool(name="sm_p", bufs=3))
        small_p = ctx.enter_context(tc.tile_pool(name="small_p", bufs=3))
        ps_p = ctx.enter_context(tc.tile_pool(name="ps_p", bufs=2, space="PSUM"))
        sum_ps = ctx.enter_context(tc.tile_pool(name="sum_ps", bufs=2, space="PSUM"))
        et_ps = ctx.enter_context(tc.tile_pool(name="et_ps", bufs=2, space="PSUM"))
        et_sb = ctx.enter_context(tc.tile_pool(name="et_sb", bufs=3))

        # ---- constants ----
        ident = const_p.tile([P, P], F32)
        ones_row = const_p.tile([1, P], F32)
        nc.vector.memset(ones_row, 1.0)
        ones_col = const_p.tile([P, 1], F32)
        nc.vector.memset(ones_col, 1.0)
        dummy = const_p.tile([P, 1], F32)

        # ---- input DMAs for W^T / dec^T, then batch-0 enc prefetch ----
        wt_sb = const_p.tile([P, 4, E], F32)  # [d within chunk j, j, e']
        nc.sync.dma_start(out=wt_sb,
                          in_=wt_d.ap().rearrange("(j p) e -> p j e", p=P))
        dect_sb = const_p.tile([P, 4, NB], F32)  # [d within chunk j, j, b]
        nc.sync.dma_start(out=dect_sb,
                          in_=dect_d.ap().rearrange("(j p) b -> p j b", p=P))
        nc.sync.dma_start(out=ident, in_=id_d.ap())

        batch_tiles = [None] * NB

        def issue_load(b):
            enc_b = enc_d.ap()[b].rearrange("(p j) e -> p j e", p=P)  # s = 32p + j
            chunks, chunks16 = [], []
            for c in range(NCHUNK):
                et = enc_p.tile([P, JPC, E], F32, tag="enc")
                if b == 0:
                    h = JPC // 2
                    nc.sync.dma_start(out=et[:, 0:h, :],
                                      in_=enc_b[:, c * JPC:c * JPC + h, :])
                    nc.sync.dma_start(out=et[:, h:JPC, :],
                                      in_=enc_b[:, c * JPC + h:(c + 1) * JPC, :])
                else:
                    nc.sync.dma_start(out=et,
                                      in_=enc_b[:, c * JPC:(c + 1) * JPC, :])
                chunks.append(et)
                if level >= 3:
                    ebt = encb_p.tile([P, JPC, E], BF16, tag="encb")
                    nc.scalar.copy(out=ebt, in_=et)
                    chunks16.append(ebt)
            batch_tiles[b] = (chunks, chunks16)

        issue_load(0)

        # ---- setup: sub = dec @ W^T rows, broadcast + transposed copies ----
        subb = []
        subT = []
        if level == 0:
            for b in range(NB):
                sb = const_p.tile([P, E], F32, tag=f"subb{b}")
                nc.vector.memset(sb, 0.01)
                subb.append(sb)
                stp = const_p.tile([P, 4], F32, tag=f"subT{b}")
                nc.vector.memset(stp, 0.01)
                subT.append(stp)
        else:
            # sub_b = dec[b] @ W^T as a [1, E] row, broadcast to all partitions
            for b in range(NB):
                sub_b_ps = set_ps.tile([1, E], F32, tag="s")
                for j in range(4):
                    nc.tensor.matmul(sub_b_ps, dect_sb[:, j, b:b + 1],
                                     wt_sb[:, j, :], start=(j == 0), stop=(j == 3))
                sub_b = set_sb.tile([1, E], F32, tag="sub_b")
                nc.vector.tensor_copy(out=sub_b, in_=sub_b_ps)
                psb = set_ps.tile([P, E], F32, tag="s")
                nc.tensor.matmul(psb, ones_row, sub_b, start=True, stop=True)
                sb = const_p.tile([P, E], F32, tag=f"subb{b}")
                nc.vector.tensor_copy(out=sb, in_=psb)
                subb.append(sb)

            # subT[b] [128, 4]: sub with e' on partitions, for the PE-path
            for b in range(NB) if OFF else []:
                stp = const_p.tile([P, 4], F32, tag=f"subT{b}")
                for ec in range(4):
                    tps = ps_p.tile([P, 1], F32, tag="sp")
                    nc.tensor.transpose(
                        tps, subb[b][0:1, ec * P:(ec + 1) * P], ident[0:1, 0:1])
                    nc.scalar.copy(out=stp[:, ec:ec + 1], in_=tps)
                subT.append(stp)

        # ---- main loop over local batches ----
        for b in range(NB):
            if b + 1 < NB:
                issue_load(b + 1)
            chunks, chunks16 = batch_tiles[b]

            # pass 1: scores[p, j] = enc_row . sub
            # DVE (fused mult+accum) for most columns, PE path for the last OFF
            scores = sm_p.tile([P, NJ], F32, tag="scores")
            for c in range(NCHUNK):
                for jj in range(JPC):
                    j = c * JPC + jj
                    if j < OFF and level >= 2:
                        continue
                    nc.vector.scalar_tensor_tensor(
                        out=dummy.broadcast_to((P, E)),
                        in0=chunks[c][:, jj, :],
                        scalar=0.0,
                        in1=subb[b],
                        op0=mybir.AluOpType.bypass,
                        op1=mybir.AluOpType.mult,
                        accum_out=scores[:, j:j + 1],
                    )
            if level >= 2:
                for j in range(OFF):
                    c, jj = j // JPC, j % JPC
                    etp = et_ps.tile([P, 4, P], F32, tag="etp")
                    for ec in range(4):
                        nc.tensor.transpose(
                            etp[:, ec, :],
                            chunks[c][:, jj, ec * P:(ec + 1) * P], ident)
                    ets = et_sb.tile([P, 4, P], F32, tag="ets")
                    nc.scalar.copy(out=ets, in_=etp)
                    scol = ps_p.tile([P, 1], F32, tag="sp")
                    for ec in range(4):
                        nc.tensor.matmul(scol, ets[:, ec, :],
                                         subT[b][:, ec:ec + 1],
                                         start=(ec == 0), stop=(ec == 3))
                    nc.scalar.copy(out=scores[:, j:j + 1], in_=scol)

            if level <= 1:
                nc.scalar.dma_start(
                    out=attn_d.ap()[b].rearrange("(p j) -> p j", p=P), in_=scores)
                continue

            # softmax: global max (free reduce, PE transpose, free reduce)
            mpart = small_p.tile([P, 1], F32, tag="mpart")
            nc.vector.tensor_reduce(mpart, scores, axis=mybir.AxisListType.X,
                                    op=mybir.AluOpType.max)
            mt_ps = ps_p.tile([1, P], F32, tag="sp")
            nc.tensor.transpose(mt_ps, mpart, ident)
            negm11 = small_p.tile([1, 1], F32, tag="negm11")
            nc.vector.tensor_reduce(negm11, mt_ps, axis=mybir.AxisListType.X,
                                    op=mybir.AluOpType.max, negate=True)
            negm_ps = ps_p.tile([P, 1], F32, tag="sp")
            nc.tensor.matmul(negm_ps, ones_row, negm11, start=True, stop=True)
            negm = small_p.tile([P, 1], F32, tag="negm")
            nc.scalar.copy(out=negm, in_=negm_ps)

            # exp(scores - max) with fused per-partition denominator
            p_sb = sm_p.tile([P, NJ], F32, tag="p")
            sumexp = small_p.tile([P, 1], F32, tag="sumexp")
            nc.scalar.activation(out=p_sb, in_=scores,
                                 func=mybir.ActivationFunctionType.Exp,
                                 bias=negm, scale=1.0, accum_out=sumexp)

            # Z = sum over partitions; rZ = 1/Z broadcast back
            z_ps = ps_p.tile([1, 1], F32, tag="sp")
            nc.tensor.matmul(z_ps, sumexp, ones_col, start=True, stop=True)
            rz11 = small_p.tile([1, 1], F32, tag="rz11")
            nc.vector.reciprocal(rz11, z_ps)
            rz_ps = ps_p.tile([P, 1], F32, tag="sp")
            nc.tensor.matmul(rz_ps, ones_row, rz11, start=True, stop=True)
            rz = small_p.tile([P, 1], F32, tag="rz")
            nc.scalar.copy(out=rz, in_=rz_ps)

            # normalized attention out
            attn_n = sm_p.tile([P, NJ], F32, tag="attn_n")
            nc.scalar.mul(attn_n, p_sb, rz)
            nc.scalar.dma_start(out=attn_d.ap()[b].rearrange("(p j) -> p j", p=P),
                                in_=attn_n)

            if level <= 2:
                continue

            # pass 2: sumResult = (sum_j exp_col_j^T @ enc_col_j) / Z on PE (bf16)
            p16 = sm_p.tile([P, NJ], BF16, tag="p16")
            nc.scalar.copy(out=p16, in_=p_sb)
            sres_ps = sum_ps.tile([1, E], F32, tag="sres")
            for c in range(NCHUNK):
                for jj in range(JPC):
                    j = c * JPC + jj
                    nc.tensor.matmul(sres_ps, p16[:, j:j + 1], chunks16[c][:, jj, :],
                                     start=(j == 0), stop=(j == NJ - 1))
            sres = small_p.tile([1, E], F32, tag="sres_sb")
            nc.scalar.activation(out=sres, in_=sres_ps,
                                 func=mybir.ActivationFunctionType.Copy,
                                 bias=0.0, scale=rz11)
            nc.scalar.dma_start(out=sum_d.ap()[b], in_=sres)

    nc.compile()
    return nc


_NC_CACHE = None


def _get_nc():
    global _NC_CACHE
    if _NC_CACHE is None:
        _NC_CACHE = build_nc()
    return _NC_CACHE


def kernel(decoderFeature, encoderFeature, W, **_ignored):
    """Full inputs in, full outputs out. Shards batch across 8 NeuronCores."""
    dec = np.ascontiguousarray(np.asarray(decoderFeature, dtype=np.float32)
                               .reshape(B, E))
    enc = np.ascontiguousarray(np.asarray(encoderFeature, dtype=np.float32))
    w = np.ascontiguousarray(np.asarray(W, dtype=np.float32))

    wt = np.ascontiguousarray(w.T)
    ident = np.eye(P, dtype=np.float32)
    nc = _get_nc()
    in_maps = []
    for c in range(NCORES):
        lo, hi = c * NB, (c + 1) * NB
        in_maps.append({
            "dect_in": np.ascontiguousarray(dec[lo:hi].T),
            "enc_in": np.ascontiguousarray(enc[lo:hi]),
            "wt_in": wt,
            "ident_in": ident,
        })
    res = run_bass_kernel_spmd(nc, in_maps, core_ids=list(range(NCORES)))
    outs = res.results

    attn = np.concatenate([outs[c]["attn_out"] for c in range(NCORES)], axis=0)
    sumr = np.concatenate([outs[c]["sum_out"] for c in range(NCORES)], axis=0)
    return attn.reshape(B, 1, S).astype(np.float32), sumr.astype(np.float32)


# revision 18
# speedup vs baseline: 1.1187x; 1.0023x over previous
"""Trainium2 Bass kernel for nn_AttentionLayer (single-query attention).

Reference computation (per batch b):
    sub  = dec[b] @ W.T                  # [512]
    s    = enc[b] @ sub                  # [4096] scores
    attn = softmax(s)                    # [4096]
    out  = attn @ enc[b]                 # [512]
Returns (attn [B,1,S], sumResult [B,E]).

Sharding: data-parallel over batch. 8 cores x 8 batches each, W replicated.
Each core streams its 64 MB of encoderFeature through SBUF exactly once
(memory-bound); scores are computed on VectorE with the fused
scalar_tensor_tensor (multiply + free-dim accumulate) op, the weighted sum
on TensorE in bf16 from a ScalarE-cast copy, softmax plumbing on
ScalarE/TensorE/VectorE.
"""

import numpy as np
from contextlib import ExitStack

import concourse.bass as bass
import concourse.bacc as bacc
import concourse.tile as tile
from concourse import mybir
from concourse.bass_utils import run_bass_kernel_spmd

F32 = mybir.dt.float32
BF16 = mybir.dt.bfloat16

# Problem shape (hardcoded; kernel.py must be self-contained).
B, S, E = 64, 4096, 512
NCORES = 8
NB = B // NCORES          # local batches per core
P = 128                   # partitions
NJ = S // P               # 32 score columns per batch
NCHUNK = 4                # DMA chunks per batch
JPC = NJ // NCHUNK        # 8 columns per chunk
OFF = 0                   # score columns per batch computed on TensorE


def build_nc(level=3):
    nc = bacc.Bacc("TRN2", target_bir_lowering=False, debug=False)

    dect_d = nc.dram_tensor("dect_in", [E, NB], F32, kind="ExternalInput")
    enc_d = nc.dram_tensor("enc_in", [NB, S, E], F32, kind="ExternalInput")
    wt_d = nc.dram_tensor("wt_in", [E, E], F32, kind="ExternalInput")
    id_d = nc.dram_tensor("ident_in", [P, P], F32, kind="ExternalInput")
    attn_d = nc.dram_tensor("attn_out", [NB, S], F32, kind="ExternalOutput")
    sum_d = nc.dram_tensor("sum_out", [NB, E], F32, kind="ExternalOutput")

    with tile.TileContext(nc) as tc, ExitStack() as ctx:
        const_p = ctx.enter_context(tc.tile_pool(name="const", bufs=1))
        set_ps = ctx.enter_context(tc.tile_pool(name="set_ps", bufs=2, space="PSUM"))
        set_sb = ctx.enter_context(tc.tile_pool(name="set_sb", bufs=2))
        enc_p = ctx.enter_context(tc.tile_pool(name="enc_p", bufs=5))
        encb_p = ctx.enter_context(tc.tile_pool(name="encb_p", bufs=2 * NCHUNK))
        sm_p = ctx.enter_context(tc.tile_pool(name="sm_p", bufs=3))
        small_p = ctx.enter_context(tc.tile_pool(name="small_p", bufs=3))
        ps_p = ctx.enter_context(tc.tile_pool(name="ps_p", bufs=4, space="PSUM"))
        sum_ps = ctx.enter_context(tc.tile_pool(name="sum_ps", bufs=2, space="PSUM"))


        # ---- constants ----
        ident = const_p.tile([P, P], F32)
        ones_row = const_p.tile([1, P], F32)
        nc.vector.memset(ones_row, 1.0)
        ones_col = const_p.tile([P, 1], F32)
        nc.vector.memset(ones_col, 1.0)
        dummy = const_p.tile([P, 1], F32)

        # ---- input DMAs for W^T / dec^T, then batch-0 enc prefetch ----
        wt_sb = const_p.tile([P, 4, E], F32)  # [d within chunk j, j, e']
        nc.sync.dma_start(out=wt_sb,
                          in_=wt_d.ap().rearrange("(j p) e -> p j e", p=P))
        dect_sb = const_p.tile([P, 4, NB], F32)  # [d within chunk j, j, b]
        nc.sync.dma_start(out=dect_sb,
                          in_=dect_d.ap().rearrange("(j p) b -> p j b", p=P))
        nc.sync.dma_start(out=ident, in_=id_d.ap())

        batch_tiles = [None] * NB

        def issue_load(b):
            enc_b = enc_d.ap()[b].rearrange("(p j) e -> p j e", p=P)  # s = 32p + j
            chunks, chunks16 = [], []
            for c in range(NCHUNK):
                et = enc_p.tile([P, JPC, E], F32, tag="enc")
                if b == 0:
                    h = JPC // 2
                    nc.sync.dma_start(out=et[:, 0:h, :],
                                      in_=enc_b[:, c * JPC:c * JPC + h, :])
                    nc.sync.dma_start(out=et[:, h:JPC, :],
                                      in_=enc_b[:, c * JPC + h:(c + 1) * JPC, :])
                else:
                    nc.sync.dma_start(out=et,
                                      in_=enc_b[:, c * JPC:(c + 1) * JPC, :])
                chunks.append(et)
                if level >= 3:
                    ebt = encb_p.tile([P, JPC, E], BF16, tag="encb")
                    nc.scalar.copy(out=ebt, in_=et)
                    chunks16.append(ebt)
            batch_tiles[b] = (chunks, chunks16)

        issue_load(0)

        # ---- setup: sub = dec @ W^T rows, broadcast + transposed copies ----
        subb = []
        subT = []
        if level == 0:
            for b in range(NB):
                sb = const_p.tile([P, E], F32, tag=f"subb{b}")
                nc.vector.memset(sb, 0.01)
                subb.append(sb)
                stp = const_p.tile([P, 4], F32, tag=f"subT{b}")
                nc.vector.memset(stp, 0.01)
                subT.append(stp)
        else:
            # sub_b = dec[b] @ W^T as a [1, E] row, broadcast to all partitions
            for b in range(NB):
                sub_b_ps = set_ps.tile([1, E], F32, tag="s")
                for j in range(4):
                    nc.tensor.matmul(sub_b_ps, dect_sb[:, j, b:b + 1],
                                     wt_sb[:, j, :], start=(j == 0), stop=(j == 3))
                sub_b = set_sb.tile([1, E], F32, tag="sub_b")
                nc.vector.tensor_copy(out=sub_b, in_=sub_b_ps)
                psb = set_ps.tile([P, E], F32, tag="s")
                nc.tensor.matmul(psb, ones_row, sub_b, start=True, stop=True)
                sb = const_p.tile([P, E], F32, tag=f"subb{b}")
                nc.vector.tensor_copy(out=sb, in_=psb)
                subb.append(sb)

            # subT[b] [128, 4]: sub with e' on partitions, for the PE-path
            for b in range(NB) if OFF else []:
                stp = const_p.tile([P, 4], F32, tag=f"subT{b}")
                for ec in range(4):
                    tps = ps_p.tile([P, 1], F32, tag="sp")
                    nc.tensor.transpose(
                        tps, subb[b][0:1, ec * P:(ec + 1) * P], ident[0:1, 0:1])
                    nc.scalar.copy(out=stp[:, ec:ec + 1], in_=tps)
                subT.append(stp)

        # ---- main loop: software-pipelined ----
        # Batch b's softmax / pass 2 are emitted interleaved with batch b+1's
        # score stream so the DVE never waits on the cross-engine softmax
        # chain, and the PE work spreads through the next DMA window.
        def softmax_a(st):
            scores = st["scores"]
            mpart = small_p.tile([P, 1], F32, tag="mpart")
            nc.vector.tensor_reduce(mpart, scores, axis=mybir.AxisListType.X,
                                    op=mybir.AluOpType.max)
            mt_ps = ps_p.tile([1, P], F32, tag="sp")
            nc.tensor.transpose(mt_ps, mpart, ident)
            negm11 = small_p.tile([1, 1], F32, tag="negm11")
            nc.vector.tensor_reduce(negm11, mt_ps, axis=mybir.AxisListType.X,
                                    op=mybir.AluOpType.max, negate=True)
            negm_ps = ps_p.tile([P, 1], F32, tag="sp")
            nc.tensor.matmul(negm_ps, ones_row, negm11, start=True, stop=True)
            negm = small_p.tile([P, 1], F32, tag="negm")
            nc.scalar.copy(out=negm, in_=negm_ps)
            p_sb = sm_p.tile([P, NJ], F32, tag="p")
            sumexp = small_p.tile([P, 1], F32, tag="sumexp")
            nc.scalar.activation(out=p_sb, in_=scores,
                                 func=mybir.ActivationFunctionType.Exp,
                                 bias=negm, scale=1.0, accum_out=sumexp)
            st["p_sb"], st["sumexp"] = p_sb, sumexp

        def softmax_b(st):
            z_ps = ps_p.tile([1, 1], F32, tag="sp")
            nc.tensor.matmul(z_ps, st["sumexp"], ones_col, start=True, stop=True)
            rz11 = small_p.tile([1, 1], F32, tag="rz11")
            nc.vector.reciprocal(rz11, z_ps)
            rz_ps = ps_p.tile([P, 1], F32, tag="sp")
            nc.tensor.matmul(rz_ps, ones_row, rz11, start=True, stop=True)
            rz = small_p.tile([P, 1], F32, tag="rz")
            nc.scalar.copy(out=rz, in_=rz_ps)
            attn_n = sm_p.tile([P, NJ], F32, tag="attn_n")
            nc.scalar.mul(attn_n, st["p_sb"], rz)
            nc.scalar.dma_start(
                out=attn_d.ap()[st["b"]].rearrange("(p j) -> p j", p=P), in_=attn_n)
            st["rz11"] = rz11

        def cast_p16(st):
            p16 = sm_p.tile([P, NJ], BF16, tag="p16")
            nc.scalar.copy(out=p16, in_=st["p_sb"])
            st["p16"] = p16

        def pass2(st):
            p16, chunks16 = st["p16"], st["chunks16"]
            sres_ps = sum_ps.tile([1, E], F32, tag="sres")
            for c in range(NCHUNK):
                for jj in range(JPC):
                    j = c * JPC + jj
                    nc.tensor.matmul(sres_ps, p16[:, j:j + 1],
                                     chunks16[c][:, jj, :],
                                     start=(j == 0), stop=(j == NJ - 1))
            sres = small_p.tile([1, E], F32, tag="sres_sb")
            nc.scalar.activation(out=sres, in_=sres_ps,
                                 func=mybir.ActivationFunctionType.Copy,
                                 bias=0.0, scale=st["rz11"])
            nc.scalar.dma_start(out=sum_d.ap()[st["b"]], in_=sres)

        pending = None
        for b in range(NB):
            if b + 1 < NB:
                issue_load(b + 1)
            chunks, chunks16 = batch_tiles[b]
            scores = sm_p.tile([P, NJ], F32, tag="scores")
            last = b == NB - 1
            for c in range(NCHUNK):
                for jj in range(JPC):
                    j = c * JPC + jj
                    nc.vector.scalar_tensor_tensor(
                        out=dummy.broadcast_to((P, E)),
                        in0=chunks[c][:, jj, :],
                        scalar=0.0,
                        in1=subb[b],
                        op0=mybir.AluOpType.bypass,
                        op1=mybir.AluOpType.mult,
                        accum_out=scores[:, j:j + 1],
                    )
                    if last and level >= 3 and j >= 12 and j % 2 == 0:
                        # HAM warm-keeper: fp32 matmul gated on the score
                        # column just produced, so the PE stays at 2.4 GHz
                        # into the final batch's weighted-sum matmuls.
                        junk_ps = set_ps.tile([1, E], F32, tag="s")
                        nc.tensor.matmul(junk_ps, scores[:, j:j + 1],
                                         chunks[c][:, jj, :],
                                         start=True, stop=True)
                if level >= 2 and pending is not None:
                    if c == 0:
                        softmax_a(pending)
                    elif c == 1:
                        softmax_b(pending)
                    elif c == 2 and level >= 3:
                        cast_p16(pending)
            if level <= 1:
                nc.scalar.dma_start(
                    out=attn_d.ap()[b].rearrange("(p j) -> p j", p=P), in_=scores)
                continue
            if pending is not None and level >= 3:
                pass2(pending)
            pending = {"b": b, "scores": scores, "chunks16": chunks16}

        if level >= 2 and pending is not None:
            softmax_a(pending)
            softmax_b(pending)
            if level >= 3:
                cast_p16(pending)
                pass2(pending)

    nc.compile()
    return nc


_NC_CACHE = None


def _get_nc():
    global _NC_CACHE
    if _NC_CACHE is None:
        _NC_CACHE = build_nc()
    return _NC_CACHE


def kernel(decoderFeature, encoderFeature, W, **_ignored):
    """Full inputs in, full outputs out. Shards batch across 8 NeuronCores."""
    dec = np.ascontiguousarray(np.asarray(decoderFeature, dtype=np.float32)
                               .reshape(B, E))
    enc = np.ascontiguousarray(np.asarray(encoderFeature, dtype=np.float32))
    w = np.ascontiguousarray(np.asarray(W, dtype=np.float32))

    wt = np.ascontiguousarray(w.T)
    ident = np.eye(P, dtype=np.float32)
    nc = _get_nc()
    in_maps = []
    for c in range(NCORES):
        lo, hi = c * NB, (c + 1) * NB
        in_maps.append({
            "dect_in": np.ascontiguousarray(dec[lo:hi].T),
            "enc_in": np.ascontiguousarray(enc[lo:hi]),
            "wt_in": wt,
            "ident_in": ident,
        })
    res = run_bass_kernel_spmd(nc, in_maps, core_ids=list(range(NCORES)))
    outs = res.results

    attn = np.concatenate([outs[c]["attn_out"] for c in range(NCORES)], axis=0)
    sumr = np.concatenate([outs[c]["sum_out"] for c in range(NCORES)], axis=0)
    return attn.reshape(B, 1, S).astype(np.float32), sumr.astype(np.float32)
